# revision 42
# baseline (speedup 1.0000x reference)
"""Trainium2 Bass kernel for nn_Attention_14370960572643 (gnn_message_passing).

Math (per batch b):
  local_pair[b,i,j,:] = local[b,i,:] + local[b,j,:]
  att  = relu(concat(local_pair, binary) @ W1 + b1)        [B,N,N,H]
  score = sigmoid(att @ W2 + b2)                            [B,N,N,1]
  G[b,i,:] = sum_j local[b,j,:] * score[b,i,j]              [B,N,H]
  outputs (E sparse pairs): lp[e] = local[bb,ii]+local[bb,jj]
                            gp[e] = G[bb,ii]+G[bb,jj]

Structure (v2):
  * att collapses to one K=112 matmul per (chunk, h-tile): contraction rows
    = 100 P rows (P = local @ W1[:H]) + 1 ones row (b1) + 11 W1b rows.  The
    moving operand packs BOTH pair indicators into the identity rows
    (rhs[r, col] = [r==j(col)] + [r==i(col)]), so the stationary C matrix is
    fully batch-constant — no per-chunk assembly at all.  fp8e4 DoubleRow
    packs K=112 into 56 partitions x 2 slabs at 0.5 cycles/col.
  * score matmuls are flipped: lhsT = att16 column block [hh, 100], rhs =
    W2 [hh, 1], out = [100, 1] psum column -> psum accumulates scoreT
    [100 j, 100 i] directly; one sigmoid per batch emits scT; G follows.
  * sparse gathers stay one-hot matmuls; outputs drain via 3-bank mega
    drains into SBUF staging, 4 large DMAs write lp/gp.
"""

import numpy as np

B, N, H, BIN = 16, 100, 300, 11
NN2 = N * N                  # 10000 pair columns per batch
NCORES = 8
BPC = B // NCORES            # batches per core
CH_I = 5                     # i values per chunk
CH = CH_I * N                # 500 pair columns per chunk
NCH = N // CH_I              # 20 chunks per batch
H_T = [(0, 128), (128, 128), (256, 44)]     # h tiles
K112 = N + 1 + BIN           # 112 contraction rows
K64 = 64                     # DoubleRow slab partitions (112 padded to 128)
WSCALE = 16.0                # W1b x16 in C, binary /16 in rhs (fp8 range)
FP8 = True

_CACHE = {}


def _build_nc(cap_b, fp8):
    import concourse.bass as bass
    import concourse.mybir as mybir
    import concourse.tile as tile
    from concourse import bacc

    dt = mybir.dt
    f32 = dt.float32
    bf16 = dt.bfloat16
    fp8t = dt.float8e4
    cdt = fp8t if fp8 else bf16
    nt_b = cap_b // 128
    cap = cap_b * BPC

    nc = bacc.Bacc("TRN2", target_bir_lowering=False, debug=False,
                   num_devices=NCORES)

    # ---- dram parameters (per-core shards) ----
    # mega-const: W1a (3x300) | localT (6x100) | lnat (2x300) | W2c (3x1)
    mc_d = nc.dram_tensor("MC", [128, 2103], bf16, kind="ExternalInput").ap()
    if fp8:
        rhs_d = nc.dram_tensor("RHS", [BPC, K64, 2, NN2], fp8t,
                               kind="ExternalInput").ap()
    else:
        rhs_d = nc.dram_tensor("RHS", [BPC, K112, NN2], bf16,
                               kind="ExternalInput").ap()
    cconst_d = nc.dram_tensor("Cconst", [1 + BIN, H], cdt,
                              kind="ExternalInput").ap()
    b2_d = nc.dram_tensor("b2", [1, 1], f32, kind="ExternalInput").ap()
    oh_d = nc.dram_tensor("oh", [N, cap], bf16, kind="ExternalInput").ap()
    lp_d = nc.dram_tensor("lp", [cap, H], bf16, kind="ExternalOutput").ap()
    gp_d = nc.dram_tensor("gp", [cap, H], bf16, kind="ExternalOutput").ap()

    Relu = mybir.ActivationFunctionType.Relu
    Sigmoid = mybir.ActivationFunctionType.Sigmoid
    DR = mybir.MatmulPerfMode.DoubleRow

    with tile.TileContext(nc) as tc:
        with (
            tc.tile_pool(name="const", bufs=1) as cpool,
            tc.tile_pool(name="attc", bufs=24) as attp,
            tc.tile_pool(name="pa", bufs=3, space="PSUM") as pa_pool,
            tc.tile_pool(name="psm", bufs=2, space="PSUM") as psm_pool,
        ):
            # ---------- SBUF constants ----------
            mc = cpool.tile([128, 2103], bf16, tag="mc", name="mc")
            nc.sync.dma_start(out=mc[:, 0:1200], in_=mc_d[:, 0:1200])
            nc.sync.dma_start(out=mc[:, 1200:2103], in_=mc_d[:, 1200:2103])
            W1a_sb = [mc[0:kk, kt * H:(kt + 1) * H]
                      for kt, (k0, kk) in enumerate(H_T)]
            localT_sb = [[mc[0:kk, 900 + (b * 3 + kt) * N:
                             900 + (b * 3 + kt + 1) * N]
                          for kt, (k0, kk) in enumerate(H_T)]
                         for b in range(BPC)]
            lnat_sb = [mc[0:N, 1500 + b * H:1500 + (b + 1) * H]
                       for b in range(BPC)]
            W2c_sb = [mc[0:hh, 2100 + kt:2101 + kt]
                      for kt, (h0, hh) in enumerate(H_T)]
            b2rep = cpool.tile([128, 1], f32, tag="b2rep", name="b2rep")
            nc.sync.dma_start(out=b2rep[:], in_=b2_d[0:1, :].to_broadcast([128, 1]))
            # dummy sigmoid at warmup pins the act table that holds
            # relu+copy+sigmoid together, avoiding mid-stream table reloads
            _junk = cpool.tile([1, 1], f32, tag="junk", name="junk")
            nc.scalar.activation(_junk[:], b2rep[0:1, :], Sigmoid)
            # PE p-state warmer: touch the PE early so the 2.4GHz ramp
            # (3us of busy history) completes before the chunk stream
            _wsb = cpool.tile([1, 8], bf16, tag="wsb", name="wsb")
            nc.vector.memset(_wsb[:], 0.0)
            _wps = psm_pool.tile([128, 512], f32, tag="m", name="wps")
            for _i in range(12):
                nc.tensor.matmul(out=_wps[0:1, 0:8], lhsT=_wsb[:, 0:1],
                                 rhs=_wsb[:], start=True, stop=True)

            RHS_sb, C_sb, scT_sb, g16_sb = [], [], [], []
            lpst, gpst = [], []
            for b in range(BPC):
                if fp8:
                    t = cpool.tile([K64, 2, NN2], fp8t, tag=f"rhs{b}",
                                   name=f"rhs{b}")
                else:
                    t = cpool.tile([K112, NN2], bf16, tag=f"rhs{b}",
                                   name=f"rhs{b}")
                RHS_sb.append(t)
                if fp8:
                    # DoubleRow lhsT must be contiguous, M >= 64: one tile
                    # per h-tile, the 44-row tail zero-padded to 64
                    c = [cpool.tile([K64, 2, max(hh, 64)], fp8t,
                                    tag=f"c{b}_{kt}", name=f"c{b}_{kt}")
                         for kt, (h0, hh) in enumerate(H_T)]
                else:
                    c = cpool.tile([K112, H], bf16, tag=f"c{b}", name=f"c{b}")
                C_sb.append(c)
                scT_sb.append(cpool.tile([N, N], bf16, tag=f"sct{b}",
                                         name=f"sct{b}"))
                g16_sb.append(cpool.tile([N, H], bf16, tag=f"g16_{b}",
                                         name=f"g16_{b}"))
                lpst.append(cpool.tile([128, nt_b, H], bf16, tag=f"lpst{b}",
                                       name=f"lpst{b}"))
                gpst.append(cpool.tile([128, nt_b, H], bf16, tag=f"gpst{b}",
                                       name=f"gpst{b}"))
            oh_sb = cpool.tile([N, cap], bf16, tag="oh", name="oh")

            def load_rhs(b, piece, npieces=4):
                w = NN2 // npieces
                sl = slice(piece * w, (piece + 1) * w)
                if fp8:
                    nc.sync.dma_start(out=RHS_sb[b][:, :, sl],
                                      in_=rhs_d[b][:, :, sl])
                else:
                    nc.sync.dma_start(out=RHS_sb[b][:, sl], in_=rhs_d[b][:, sl])

            def load_cconst(b):
                if fp8:
                    # rows 100..111 = slab1 partitions 36..47; zero-fill the
                    # pad rows 48..63 first (whole slab1, then overwritten)
                    for kt, (h0, hh) in enumerate(H_T):
                        if hh < 64:
                            nc.gpsimd.memset(C_sb[b][kt][:, 0, :], 0.0)
                        nc.gpsimd.memset(C_sb[b][kt][:, 1, :], 0.0)
                        nc.sync.dma_start(out=C_sb[b][kt][36:48, 1, 0:hh],
                                          in_=cconst_d[:, h0:h0 + hh])
                else:
                    nc.sync.dma_start(out=C_sb[b][N:K112, :], in_=cconst_d[:])

            def p_stage(b):
                psm = psm_pool.tile([128, 512], f32, tag="m", name=f"psp{b}")
                ps = psm[0:N, 0:H]
                for kt in range(3):
                    nc.tensor.matmul(out=ps[:], lhsT=localT_sb[b][kt][:],
                                     rhs=W1a_sb[kt][:],
                                     start=(kt == 0), stop=(kt == 2))
                if fp8:
                    for kt, (h0, hh) in enumerate(H_T):
                        nc.vector.tensor_copy(out=C_sb[b][kt][0:64, 0, 0:hh],
                                              in_=ps[0:64, h0:h0 + hh])
                        nc.scalar.copy(out=C_sb[b][kt][0:36, 1, 0:hh],
                                       in_=ps[64:100, h0:h0 + hh])
                else:
                    nc.vector.tensor_copy(out=C_sb[b][0:N, :], in_=ps[:])

            # drain engine rotation: index 0=DVE, 1=ACT, 2=Pool
            def drain(eng, out_ap, in_ap):
                if eng == 0:
                    nc.vector.tensor_scalar_max(out=out_ap, in0=in_ap,
                                                scalar1=0.0)
                elif eng == 1:
                    nc.scalar.activation(out_ap, in_ap, Relu)
                else:
                    nc.gpsimd.tensor_scalar_max(out=out_ap, in0=in_ap,
                                                scalar1=0.0)

            def copy_drain(eng, out_ap, in_ap):
                if eng == 0:
                    nc.vector.tensor_copy(out=out_ap, in_=in_ap)
                elif eng == 1:
                    nc.scalar.copy(out=out_ap, in_=in_ap)
                else:
                    nc.gpsimd.tensor_copy(out=out_ap, in_=in_ap)

            # GPSIMD cannot touch PSUM: drains go to DVE/ACT, assigned
            # greedily by estimated accumulated engine time.
            HCH = CH // 2          # half-chunk columns (250)
            GH = H // 2            # gather H-half (150)
            ROT = [1, 0, 1, 0, 1, 0, 1, 0, 1, 0, 1, 0, 1, 0, 1, 0, 1]
            state = {"rot": 0}

            def next_eng(free_sz):
                e = ROT[state["rot"] % len(ROT)]
                state["rot"] += 1
                return e

            def emit_att_half(b, ic, h, attc):
                pa = pa_pool.tile([128, 3, 256], f32, tag="a",
                                  name=f"pa{b}_{ic}_{h}")
                c0 = ic * CH + h * HCH
                for kt, (h0, hh) in enumerate(H_T):
                    if fp8:
                        nc.tensor.matmul(
                            out=pa[0:max(hh, 64), kt, 0:HCH],
                            lhsT=C_sb[b][kt][:],
                            rhs=RHS_sb[b][:, :, c0:c0 + HCH],
                            start=True, stop=True, perf_mode=DR)
                    else:
                        nc.tensor.matmul(
                            out=pa[0:hh, kt, 0:HCH],
                            lhsT=C_sb[b][:, h0:h0 + hh],
                            rhs=RHS_sb[b][:, c0:c0 + HCH],
                            start=True, stop=True)
                out_ap = attc[:, :].rearrange(
                    "p (t c) -> p t c", t=3)[:, :, h * HCH:(h + 1) * HCH]
                drain(next_eng(3 * HCH), out_ap, pa[:, :, 0:HCH])

            def emit_gather_unit(b, rhs_sb, stage, blocks):
                # blocks: consecutive H-half indices (block = tile*2 + half)
                pa = pa_pool.tile([128, 3, 256], f32, tag="a",
                                  name=f"pg{b}_{blocks[0]}_{len(blocks)}")
                for q, blk in enumerate(blocks):
                    t_i, hf = blk // 2, blk % 2
                    col0 = b * cap_b + t_i * 128
                    nc.tensor.matmul(out=pa[0:128, q, 0:GH],
                                     lhsT=oh_sb[:, col0:col0 + 128],
                                     rhs=rhs_sb[:, hf * GH:(hf + 1) * GH],
                                     start=True, stop=True)
                nq = len(blocks)
                st = stage[:].rearrange("p t (u c) -> p (t u) c", u=2)
                copy_drain(next_eng(nq * GH),
                           st[:, blocks[0]:blocks[0] + nq, :],
                           pa[:, 0:nq, 0:GH])

            def emit_score_group(b, psc, ic, attc, s):
                i = ic * CH_I + s
                for kt, (h0, hh) in enumerate(H_T):
                    nc.tensor.matmul(
                        out=psc[0:N, i:i + 1],
                        lhsT=attc[0:hh, kt * CH + s * N:kt * CH + (s + 1) * N],
                        rhs=W2c_sb[kt][:],
                        start=(kt == 0), stop=(kt == 2))

            def emit_score(b, psc, ic, attc):
                for s in range(CH_I):
                    emit_score_group(b, psc, ic, attc, s)

            def emit_sig_g(b, psc):
                nc.scalar.activation(scT_sb[b][:], psc[0:N, 0:N], Sigmoid,
                                     bias=b2rep[0:N, :])
                psm = psm_pool.tile([128, 512], f32, tag="m", name=f"psg{b}")
                nc.tensor.matmul(out=psm[0:N, 0:H], lhsT=scT_sb[b][:],
                                 rhs=lnat_sb[b][:], start=True, stop=True)
                nc.vector.tensor_copy(out=g16_sb[b][:], in_=psm[0:N, 0:H])

            def emit_out_dma(d, b, stage):
                nc.scalar.dma_start(
                    out=d[b * cap_b:(b + 1) * cap_b, :].rearrange(
                        "(t p) h -> p t h", p=128),
                    in_=stage[:])

            def gather_units(b, which):
                # which: 0 = lp (rhs lnat), 1 = gp (rhs g16)
                rhs_sb = lnat_sb[b] if which == 0 else g16_sb[b]
                stage = lpst[b] if which == 0 else gpst[b]
                nblk = 2 * nt_b
                units = []
                for g0 in range(0, nblk, 3):
                    blocks = list(range(g0, min(g0 + 3, nblk)))
                    units.append(("g", b, rhs_sb, stage, blocks))
                return units

            def run_phase(b, extra_units, final_extras):
                """Interleave this batch's att half-chunks with extra
                (gather/deferred) units; lag scores behind the drains.
                Allocates this batch's score psum here so the tag-m bank
                rotation is P0, sc0, P1, sc1, G0, G1 (no live overlap)."""
                psc = psm_pool.tile([128, 512], f32, tag="m", name=f"sc{b}")
                LAG = 18          # units of lag before scores start
                cadence = 2 if len(extra_units) > 10 else 4
                squeue = []      # (ic, attc, s) score groups not yet emitted
                extra = list(extra_units)
                attc = None
                ucount = 0
                for ic in range(NCH):
                    attc = attp.tile([128, 3 * CH], bf16, tag="attc",
                                     name=f"attc{b}_{ic}")
                    for hh_i in range(2):
                        emit_att_half(b, ic, hh_i, attc)
                        ucount += 1
                        if ucount % cadence == 0 and ucount >= 8 and extra:
                            u = extra.pop(0)
                            if u[0] == "g":
                                emit_gather_unit(u[1], u[2], u[3], u[4])
                            else:
                                u[1]()
                        # drain up to 3 score groups per unit, lagged
                        nready = (ucount - LAG) * 5 // 2
                        ndone = ic * 2 * CH_I + hh_i * CH_I  # upper bound
                        while squeue and len(squeue) > max(
                                0, ndone - nready):
                            g = squeue.pop(0)
                            emit_score_group(b, psc, *g)
                    for s in range(CH_I):
                        squeue.append((ic, attc, s))
                for u in extra:
                    if u[0] == "g":
                        emit_gather_unit(u[1], u[2], u[3], u[4])
                    else:
                        u[1]()
                for g in squeue:
                    emit_score_group(b, psc, *g)
                emit_sig_g(b, psc)
                for fu in final_extras:
                    fu()

            # ---------------- schedule ----------------
            # DMA order matters: HWDGE serializes ~640ns per DMA.  Critical
            # path first: mega-const (P-stage), first RHS piece, Cconst b0.
            load_rhs(0, 0)
            load_cconst(0)
            p_stage(0)
            load_rhs(0, 1)
            nc.sync.dma_start(out=oh_sb[:], in_=oh_d[:])
            load_rhs(0, 2)
            load_rhs(0, 3)
            load_cconst(1)
            for p in range(2):
                w = NN2 // 2
                sl = slice(p * w, (p + 1) * w)
                if fp8:
                    nc.sync.dma_start(out=RHS_sb[1][:, :, sl],
                                      in_=rhs_d[1][:, :, sl])
                else:
                    nc.sync.dma_start(out=RHS_sb[1][:, sl], in_=rhs_d[1][:, sl])

            # b0 phase: att b0 + lp-b0 gathers + P-stage for b1
            run_phase(0,
                      [("p", lambda: p_stage(1))] + gather_units(0, 0),
                      [lambda: emit_out_dma(lp_d, 0, lpst[0])])
            # b1 phase: att b1 + lp-b1 + gp-b0 gathers
            run_phase(1,
                      gather_units(1, 0) + gather_units(0, 1),
                      [lambda: emit_out_dma(lp_d, 1, lpst[1]),
                       lambda: emit_out_dma(gp_d, 0, gpst[0])])
            # tail: gp-b1; split the out-DMA so the first half overlaps
            # the remaining drains
            tail_units = gather_units(1, 1)
            nsplit = len(tail_units) // 2
            for u in tail_units[:nsplit]:
                emit_gather_unit(u[1], u[2], u[3], u[4])
            t_half = (tail_units[nsplit][4][0] // 2) * 128
            nc.scalar.dma_start(
                out=gp_d[cap_b:cap_b + t_half, :].rearrange(
                    "(t p) h -> p t h", p=128),
                in_=gpst[1][:, 0:t_half // 128, :])
            for u in tail_units[nsplit:]:
                emit_gather_unit(u[1], u[2], u[3], u[4])
            nc.scalar.dma_start(
                out=gp_d[cap_b + t_half:2 * cap_b, :].rearrange(
                    "(t p) h -> p t h", p=128),
                in_=gpst[1][:, t_half // 128:, :])

    nc.compile()
    return nc


def _prep_inputs(local_feats, binary_feats, sparse_idx, W1, b1, W2, b2):
    """Build per-core in_maps + reassembly info. Host-side layout only."""
    import ml_dtypes
    bf = ml_dtypes.bfloat16
    f8 = ml_dtypes.float8_e4m3
    cdt = f8 if FP8 else bf
    local_feats = np.ascontiguousarray(local_feats, dtype=np.float32)
    binary_feats = np.ascontiguousarray(binary_feats, dtype=np.float32)
    sparse_idx = np.asarray(sparse_idx)
    W1 = np.ascontiguousarray(W1, dtype=np.float32)
    b1 = np.ascontiguousarray(b1, dtype=np.float32).reshape(1, H)
    W2 = np.ascontiguousarray(W2, dtype=np.float32).reshape(H, 1)
    b2 = np.ascontiguousarray(b2, dtype=np.float32).reshape(1, 1)

    bb = sparse_idx[:, 0].astype(np.int64)
    ii = sparse_idx[:, 1].astype(np.int64)
    jj = sparse_idx[:, 2].astype(np.int64)

    # runtime-sized sparse capacity per (core, batch)
    counts = np.bincount(bb, minlength=B)
    cap_b = max(128, int(-(-counts.max() // 128)) * 128)
    cap = cap_b * BPC

    # IND2: rows 0..99 = [r==j]+[r==i]; row 100 = ones (b1 row)
    cols = np.arange(NN2)
    ind2 = np.zeros((N + 1, NN2), dtype=np.float32)
    np.add.at(ind2, (cols % N, cols), 1.0)
    np.add.at(ind2, (cols // N, cols), 1.0)
    ind2[N, :] = 1.0

    cconst = np.concatenate([b1, W1[H:] * WSCALE], axis=0).astype(cdt)

    in_maps, pos_list = [], []
    for c in range(NCORES):
        sl = slice(c * BPC, c * BPC + BPC)
        binT = binary_feats[sl].transpose(0, 3, 1, 2).reshape(BPC, BIN, NN2)
        rhs112 = np.concatenate(
            [np.broadcast_to(ind2, (BPC, N + 1, NN2)), binT / WSCALE], axis=1)
        if FP8:
            rhs128 = np.concatenate(
                [rhs112, np.zeros((BPC, 16, NN2), np.float32)], axis=1)
            rhs = np.ascontiguousarray(
                rhs128.reshape(BPC, 2, 64, NN2).transpose(0, 2, 1, 3)
            ).astype(f8)
        else:
            rhs = np.ascontiguousarray(rhs112).astype(bf)

        oh = np.zeros((N, cap), dtype=np.float32)
        pos_c = []
        for b in range(BPC):
            gb = c * BPC + b
            pos = np.nonzero(bb == gb)[0]
            assert len(pos) <= cap_b
            colsb = b * cap_b + np.arange(len(pos))
            np.add.at(oh, (ii[pos], colsb), 1.0)
            np.add.at(oh, (jj[pos], colsb), 1.0)
            pos_c.append(pos)
        mc = np.zeros((128, 2103), dtype=np.float32)
        localT = local_feats[sl].transpose(0, 2, 1)          # [BPC, H, N]
        for kt, (k0, kk) in enumerate(((0, 128), (128, 128), (256, 44))):
            mc[0:kk, kt * H:(kt + 1) * H] = W1[k0:k0 + kk, :H]
            for b in range(BPC):
                mc[0:kk, 900 + (b * 3 + kt) * N:900 + (b * 3 + kt + 1) * N] = \
                    localT[b, k0:k0 + kk, :]
            mc[0:kk, 2100 + kt] = W2[k0:k0 + kk, 0]
        for b in range(BPC):
            mc[0:N, 1500 + b * H:1500 + (b + 1) * H] = \
                local_feats[sl][b].reshape(N, H)
        in_maps.append({
            "MC": mc.astype(bf),
            "RHS": rhs,
            "Cconst": cconst,
            "b2": b2,
            "oh": oh.astype(bf),
        })
        pos_list.append(pos_c)
    return in_maps, pos_list, cap_b


def _run(in_maps, cap_b, trace=False):
    from concourse.bass_utils import run_bass_kernel_spmd
    key = (cap_b, FP8)
    if key not in _CACHE:
        _CACHE[key] = _build_nc(cap_b, FP8)
    nc = _CACHE[key]
    _CACHE["last_nc"] = nc
    res = run_bass_kernel_spmd(nc, in_maps, core_ids=list(range(NCORES)),
                               trace=trace)
    return res


def kernel(local_feats, binary_feats, sparse_idx, W1, b1, W2, b2):
    in_maps, pos_list, cap_b = _prep_inputs(
        local_feats, binary_feats, sparse_idx, W1, b1, W2, b2)
    res = _run(in_maps, cap_b)
    E = sparse_idx.shape[0]
    lp_full = np.zeros((E, H), dtype=np.float32)
    gp_full = np.zeros((E, H), dtype=np.float32)
    for c in range(NCORES):
        for b in range(BPC):
            pos = pos_list[c][b]
            r0 = b * cap_b
            lp_full[pos] = res.results[c]["lp"][r0:r0 + len(pos)].astype(
                np.float32)
            gp_full[pos] = res.results[c]["gp"][r0:r0 + len(pos)].astype(
                np.float32)
    return (lp_full, gp_full)


# revision 43
# speedup vs baseline: 1.0042x; 1.0042x over previous
"""Trainium2 Bass kernel for nn_Attention_14370960572643 (gnn_message_passing).

Math (per batch b):
  local_pair[b,i,j,:] = local[b,i,:] + local[b,j,:]
  att  = relu(concat(local_pair, binary) @ W1 + b1)        [B,N,N,H]
  score = sigmoid(att @ W2 + b2)                            [B,N,N,1]
  G[b,i,:] = sum_j local[b,j,:] * score[b,i,j]              [B,N,H]
  outputs (E sparse pairs): lp[e] = local[bb,ii]+local[bb,jj]
                            gp[e] = G[bb,ii]+G[bb,jj]

Structure (v2):
  * att collapses to one K=112 matmul per (chunk, h-tile): contraction rows
    = 100 P rows (P = local @ W1[:H]) + 1 ones row (b1) + 11 W1b rows.  The
    moving operand packs BOTH pair indicators into the identity rows
    (rhs[r, col] = [r==j(col)] + [r==i(col)]), so the stationary C matrix is
    fully batch-constant — no per-chunk assembly at all.  fp8e4 DoubleRow
    packs K=112 into 56 partitions x 2 slabs at 0.5 cycles/col.
  * score matmuls are flipped: lhsT = att16 column block [hh, 100], rhs =
    W2 [hh, 1], out = [100, 1] psum column -> psum accumulates scoreT
    [100 j, 100 i] directly; one sigmoid per batch emits scT; G follows.
  * sparse gathers stay one-hot matmuls; outputs drain via 3-bank mega
    drains into SBUF staging, 4 large DMAs write lp/gp.
"""

import numpy as np

B, N, H, BIN = 16, 100, 300, 11
NN2 = N * N                  # 10000 pair columns per batch
NCORES = 8
BPC = B // NCORES            # batches per core
CH_I = 5                     # i values per chunk
CH = CH_I * N                # 500 pair columns per chunk
NCH = N // CH_I              # 20 chunks per batch
H_T = [(0, 128), (128, 128), (256, 44)]     # h tiles
K112 = N + 1 + BIN           # 112 contraction rows
K64 = 64                     # DoubleRow slab partitions (112 padded to 128)
WSCALE = 16.0                # W1b x16 in C, binary /16 in rhs (fp8 range)
FP8 = True

_CACHE = {}


def _build_nc(cap_b, fp8):
    import concourse.bass as bass
    import concourse.mybir as mybir
    import concourse.tile as tile
    from concourse import bacc

    dt = mybir.dt
    f32 = dt.float32
    bf16 = dt.bfloat16
    fp8t = dt.float8e4
    cdt = fp8t if fp8 else bf16
    nt_b = cap_b // 128
    cap = cap_b * BPC

    nc = bacc.Bacc("TRN2", target_bir_lowering=False, debug=False,
                   num_devices=NCORES)

    # ---- dram parameters (per-core shards) ----
    # mega-const: W1a (3x300) | localT (6x100) | lnat (2x300) | W2c (3x1)
    mc_d = nc.dram_tensor("MC", [128, 2103], bf16, kind="ExternalInput").ap()
    if fp8:
        rhs_d = nc.dram_tensor("RHS", [BPC, K64, 2, NN2], fp8t,
                               kind="ExternalInput").ap()
    else:
        rhs_d = nc.dram_tensor("RHS", [BPC, K112, NN2], bf16,
                               kind="ExternalInput").ap()
    cconst_d = nc.dram_tensor("Cconst", [1 + BIN, H], cdt,
                              kind="ExternalInput").ap()
    b2_d = nc.dram_tensor("b2", [1, 1], f32, kind="ExternalInput").ap()
    oh_d = nc.dram_tensor("oh", [N, cap], bf16, kind="ExternalInput").ap()
    lp_d = nc.dram_tensor("lp", [cap, H], bf16, kind="ExternalOutput").ap()
    gp_d = nc.dram_tensor("gp", [cap, H], bf16, kind="ExternalOutput").ap()

    Relu = mybir.ActivationFunctionType.Relu
    Sigmoid = mybir.ActivationFunctionType.Sigmoid
    DR = mybir.MatmulPerfMode.DoubleRow

    with tile.TileContext(nc) as tc:
        with (
            tc.tile_pool(name="const", bufs=1) as cpool,
            tc.tile_pool(name="attc", bufs=24) as attp,
            tc.tile_pool(name="pa", bufs=3, space="PSUM") as pa_pool,
            tc.tile_pool(name="psm", bufs=2, space="PSUM") as psm_pool,
        ):
            # ---------- SBUF constants ----------
            mc = cpool.tile([128, 2103], bf16, tag="mc", name="mc")
            nc.sync.dma_start(out=mc[:, 0:1200], in_=mc_d[:, 0:1200])
            nc.sync.dma_start(out=mc[:, 1200:2103], in_=mc_d[:, 1200:2103])
            W1a_sb = [mc[0:kk, kt * H:(kt + 1) * H]
                      for kt, (k0, kk) in enumerate(H_T)]
            localT_sb = [[mc[0:kk, 900 + (b * 3 + kt) * N:
                             900 + (b * 3 + kt + 1) * N]
                          for kt, (k0, kk) in enumerate(H_T)]
                         for b in range(BPC)]
            lnat_sb = [mc[0:N, 1500 + b * H:1500 + (b + 1) * H]
                       for b in range(BPC)]
            W2c_sb = [mc[0:hh, 2100 + kt:2101 + kt]
                      for kt, (h0, hh) in enumerate(H_T)]
            b2rep = cpool.tile([128, 1], f32, tag="b2rep", name="b2rep")
            nc.sync.dma_start(out=b2rep[:], in_=b2_d[0:1, :].to_broadcast([128, 1]))
            # dummy sigmoid at warmup pins the act table that holds
            # relu+copy+sigmoid together, avoiding mid-stream table reloads
            _junk = cpool.tile([1, 1], f32, tag="junk", name="junk")
            nc.scalar.activation(_junk[:], b2rep[0:1, :], Sigmoid)
            # PE p-state warmer: touch the PE early so the 2.4GHz ramp
            # (3us of busy history) completes before the chunk stream
            _wsb = cpool.tile([1, 8], bf16, tag="wsb", name="wsb")
            nc.vector.memset(_wsb[:], 0.0)
            _wps = psm_pool.tile([128, 512], f32, tag="m", name="wps")
            for _i in range(12):
                nc.tensor.matmul(out=_wps[0:1, 0:8], lhsT=_wsb[:, 0:1],
                                 rhs=_wsb[:], start=True, stop=True)

            RHS_sb, C_sb, scT_sb, g16_sb = [], [], [], []
            lpst, gpst = [], []
            for b in range(BPC):
                if fp8:
                    t = cpool.tile([K64, 2, NN2], fp8t, tag=f"rhs{b}",
                                   name=f"rhs{b}")
                else:
                    t = cpool.tile([K112, NN2], bf16, tag=f"rhs{b}",
                                   name=f"rhs{b}")
                RHS_sb.append(t)
                if fp8:
                    # DoubleRow lhsT must be contiguous, M >= 64: one tile
                    # per h-tile, the 44-row tail zero-padded to 64
                    c = [cpool.tile([K64, 2, max(hh, 64)], fp8t,
                                    tag=f"c{b}_{kt}", name=f"c{b}_{kt}")
                         for kt, (h0, hh) in enumerate(H_T)]
                else:
                    c = cpool.tile([K112, H], bf16, tag=f"c{b}", name=f"c{b}")
                C_sb.append(c)
                scT_sb.append(cpool.tile([N, N], bf16, tag=f"sct{b}",
                                         name=f"sct{b}"))
                g16_sb.append(cpool.tile([N, H], bf16, tag=f"g16_{b}",
                                         name=f"g16_{b}"))
                lpst.append(cpool.tile([128, nt_b, H], bf16, tag=f"lpst{b}",
                                       name=f"lpst{b}"))
                gpst.append(cpool.tile([128, nt_b, H], bf16, tag=f"gpst{b}",
                                       name=f"gpst{b}"))
            oh_sb = cpool.tile([N, cap], bf16, tag="oh", name="oh")

            def load_rhs(b, piece, npieces=4):
                w = NN2 // npieces
                sl = slice(piece * w, (piece + 1) * w)
                if fp8:
                    nc.sync.dma_start(out=RHS_sb[b][:, :, sl],
                                      in_=rhs_d[b][:, :, sl])
                else:
                    nc.sync.dma_start(out=RHS_sb[b][:, sl], in_=rhs_d[b][:, sl])

            def load_cconst(b):
                if fp8:
                    # rows 100..111 = slab1 partitions 36..47; zero-fill the
                    # pad rows 48..63 first (whole slab1, then overwritten)
                    for kt, (h0, hh) in enumerate(H_T):
                        if hh < 64:
                            nc.gpsimd.memset(C_sb[b][kt][:, 0, :], 0.0)
                        nc.gpsimd.memset(C_sb[b][kt][:, 1, :], 0.0)
                        nc.sync.dma_start(out=C_sb[b][kt][36:48, 1, 0:hh],
                                          in_=cconst_d[:, h0:h0 + hh])
                else:
                    nc.sync.dma_start(out=C_sb[b][N:K112, :], in_=cconst_d[:])

            def p_stage(b):
                psm = psm_pool.tile([128, 512], f32, tag="m", name=f"psp{b}")
                ps = psm[0:N, 0:H]
                for kt in range(3):
                    nc.tensor.matmul(out=ps[:], lhsT=localT_sb[b][kt][:],
                                     rhs=W1a_sb[kt][:],
                                     start=(kt == 0), stop=(kt == 2))
                if fp8:
                    for kt, (h0, hh) in enumerate(H_T):
                        nc.vector.tensor_copy(out=C_sb[b][kt][0:64, 0, 0:hh],
                                              in_=ps[0:64, h0:h0 + hh])
                        nc.scalar.copy(out=C_sb[b][kt][0:36, 1, 0:hh],
                                       in_=ps[64:100, h0:h0 + hh])
                else:
                    nc.vector.tensor_copy(out=C_sb[b][0:N, :], in_=ps[:])

            # drain engine rotation: index 0=DVE, 1=ACT, 2=Pool
            def drain(eng, out_ap, in_ap):
                if eng == 0:
                    nc.vector.tensor_scalar_max(out=out_ap, in0=in_ap,
                                                scalar1=0.0)
                elif eng == 1:
                    nc.scalar.activation(out_ap, in_ap, Relu)
                else:
                    nc.gpsimd.tensor_scalar_max(out=out_ap, in0=in_ap,
                                                scalar1=0.0)

            def copy_drain(eng, out_ap, in_ap):
                if eng == 0:
                    nc.vector.tensor_copy(out=out_ap, in_=in_ap)
                elif eng == 1:
                    nc.scalar.copy(out=out_ap, in_=in_ap)
                else:
                    nc.gpsimd.tensor_copy(out=out_ap, in_=in_ap)

            # GPSIMD cannot touch PSUM: drains go to DVE/ACT, assigned
            # greedily by estimated accumulated engine time.
            HCH = CH // 2          # half-chunk columns (250)
            GH = H // 2            # gather H-half (150)
            ROT = [1, 0, 1, 0, 1, 0, 1, 0, 1, 0, 1, 0, 1, 0, 1, 0, 1]
            state = {"rot": 0}

            def next_eng(free_sz):
                e = ROT[state["rot"] % len(ROT)]
                state["rot"] += 1
                return e

            def emit_att_half(b, ic, h, attc):
                pa = pa_pool.tile([128, 3, 256], f32, tag="a",
                                  name=f"pa{b}_{ic}_{h}")
                c0 = ic * CH + h * HCH
                for kt, (h0, hh) in enumerate(H_T):
                    if fp8:
                        nc.tensor.matmul(
                            out=pa[0:max(hh, 64), kt, 0:HCH],
                            lhsT=C_sb[b][kt][:],
                            rhs=RHS_sb[b][:, :, c0:c0 + HCH],
                            start=True, stop=True, perf_mode=DR)
                    else:
                        nc.tensor.matmul(
                            out=pa[0:hh, kt, 0:HCH],
                            lhsT=C_sb[b][:, h0:h0 + hh],
                            rhs=RHS_sb[b][:, c0:c0 + HCH],
                            start=True, stop=True)
                out_ap = attc[:, :].rearrange(
                    "p (t c) -> p t c", t=3)[:, :, h * HCH:(h + 1) * HCH]
                drain(next_eng(3 * HCH), out_ap, pa[:, :, 0:HCH])

            def emit_gather_unit(b, rhs_sb, stage, blocks):
                # blocks: consecutive H-half indices (block = tile*2 + half)
                pa = pa_pool.tile([128, 3, 256], f32, tag="a",
                                  name=f"pg{b}_{blocks[0]}_{len(blocks)}")
                for q, blk in enumerate(blocks):
                    t_i, hf = blk // 2, blk % 2
                    col0 = b * cap_b + t_i * 128
                    nc.tensor.matmul(out=pa[0:128, q, 0:GH],
                                     lhsT=oh_sb[:, col0:col0 + 128],
                                     rhs=rhs_sb[:, hf * GH:(hf + 1) * GH],
                                     start=True, stop=True)
                nq = len(blocks)
                st = stage[:].rearrange("p t (u c) -> p (t u) c", u=2)
                copy_drain(next_eng(nq * GH),
                           st[:, blocks[0]:blocks[0] + nq, :],
                           pa[:, 0:nq, 0:GH])

            def emit_score_group(b, psc, ic, attc, s):
                i = ic * CH_I + s
                for kt, (h0, hh) in enumerate(H_T):
                    nc.tensor.matmul(
                        out=psc[0:N, i:i + 1],
                        lhsT=attc[0:hh, kt * CH + s * N:kt * CH + (s + 1) * N],
                        rhs=W2c_sb[kt][:],
                        start=(kt == 0), stop=(kt == 2))

            def emit_score(b, psc, ic, attc):
                for s in range(CH_I):
                    emit_score_group(b, psc, ic, attc, s)

            def emit_sig_g(b, psc):
                nc.scalar.activation(scT_sb[b][:], psc[0:N, 0:N], Sigmoid,
                                     bias=b2rep[0:N, :])
                psm = psm_pool.tile([128, 512], f32, tag="m", name=f"psg{b}")
                nc.tensor.matmul(out=psm[0:N, 0:H], lhsT=scT_sb[b][:],
                                 rhs=lnat_sb[b][:], start=True, stop=True)
                nc.vector.tensor_copy(out=g16_sb[b][:], in_=psm[0:N, 0:H])

            def emit_out_dma(d, b, stage):
                nc.scalar.dma_start(
                    out=d[b * cap_b:(b + 1) * cap_b, :].rearrange(
                        "(t p) h -> p t h", p=128),
                    in_=stage[:])

            def gather_units(b, which):
                # which: 0 = lp (rhs lnat), 1 = gp (rhs g16)
                rhs_sb = lnat_sb[b] if which == 0 else g16_sb[b]
                stage = lpst[b] if which == 0 else gpst[b]
                nblk = 2 * nt_b
                units = []
                for g0 in range(0, nblk, 3):
                    blocks = list(range(g0, min(g0 + 3, nblk)))
                    units.append(("g", b, rhs_sb, stage, blocks))
                return units

            def run_phase(b, extra_units, final_extras):
                """Interleave this batch's att half-chunks with extra
                (gather/deferred) units; lag scores behind the drains.
                Allocates this batch's score psum here so the tag-m bank
                rotation is P0, sc0, P1, sc1, G0, G1 (no live overlap)."""
                psc = psm_pool.tile([128, 512], f32, tag="m", name=f"sc{b}")
                LAG = 18          # units of lag before scores start
                cadence = 2 if len(extra_units) > 10 else 4
                squeue = []      # (ic, attc, s) score groups not yet emitted
                extra = list(extra_units)
                attc = None
                ucount = 0
                for ic in range(NCH):
                    attc = attp.tile([128, 3 * CH], bf16, tag="attc",
                                     name=f"attc{b}_{ic}")
                    for hh_i in range(2):
                        emit_att_half(b, ic, hh_i, attc)
                        ucount += 1
                        if ucount % cadence == 0 and ucount >= 10 and extra:
                            u = extra.pop(0)
                            if u[0] == "g":
                                emit_gather_unit(u[1], u[2], u[3], u[4])
                            else:
                                u[1]()
                        # drain up to 3 score groups per unit, lagged
                        nready = (ucount - LAG) * 5 // 2
                        ndone = ic * 2 * CH_I + hh_i * CH_I  # upper bound
                        while squeue and len(squeue) > max(
                                0, ndone - nready):
                            g = squeue.pop(0)
                            emit_score_group(b, psc, *g)
                    for s in range(CH_I):
                        squeue.append((ic, attc, s))
                for u in extra:
                    if u[0] == "g":
                        emit_gather_unit(u[1], u[2], u[3], u[4])
                    else:
                        u[1]()
                for g in squeue:
                    emit_score_group(b, psc, *g)
                emit_sig_g(b, psc)
                for fu in final_extras:
                    fu()

            # ---------------- schedule ----------------
            # DMA order matters: HWDGE serializes ~640ns per DMA.  Critical
            # path first: mega-const (P-stage), first RHS piece, Cconst b0.
            load_rhs(0, 0)
            load_cconst(0)
            p_stage(0)
            load_rhs(0, 1)
            nc.sync.dma_start(out=oh_sb[:], in_=oh_d[:])
            load_rhs(0, 2)
            load_rhs(0, 3)
            load_cconst(1)
            for p in range(2):
                w = NN2 // 2
                sl = slice(p * w, (p + 1) * w)
                if fp8:
                    nc.sync.dma_start(out=RHS_sb[1][:, :, sl],
                                      in_=rhs_d[1][:, :, sl])
                else:
                    nc.sync.dma_start(out=RHS_sb[1][:, sl], in_=rhs_d[1][:, sl])

            # b0 phase: att b0 + lp-b0 gathers + P-stage for b1
            run_phase(0,
                      [("p", lambda: p_stage(1))] + gather_units(0, 0),
                      [lambda: emit_out_dma(lp_d, 0, lpst[0])])
            # b1 phase: att b1 + lp-b1 + gp-b0 gathers
            run_phase(1,
                      gather_units(1, 0) + gather_units(0, 1),
                      [lambda: emit_out_dma(lp_d, 1, lpst[1]),
                       lambda: emit_out_dma(gp_d, 0, gpst[0])])
            # tail: gp-b1; split the out-DMA so the first half overlaps
            # the remaining drains
            tail_units = gather_units(1, 1)
            nsplit = len(tail_units) // 2
            for u in tail_units[:nsplit]:
                emit_gather_unit(u[1], u[2], u[3], u[4])
            t_half = (tail_units[nsplit][4][0] // 2) * 128
            nc.scalar.dma_start(
                out=gp_d[cap_b:cap_b + t_half, :].rearrange(
                    "(t p) h -> p t h", p=128),
                in_=gpst[1][:, 0:t_half // 128, :])
            for u in tail_units[nsplit:]:
                emit_gather_unit(u[1], u[2], u[3], u[4])
            nc.scalar.dma_start(
                out=gp_d[cap_b + t_half:2 * cap_b, :].rearrange(
                    "(t p) h -> p t h", p=128),
                in_=gpst[1][:, t_half // 128:, :])

    nc.compile()
    return nc


def _prep_inputs(local_feats, binary_feats, sparse_idx, W1, b1, W2, b2):
    """Build per-core in_maps + reassembly info. Host-side layout only."""
    import ml_dtypes
    bf = ml_dtypes.bfloat16
    f8 = ml_dtypes.float8_e4m3
    cdt = f8 if FP8 else bf
    local_feats = np.ascontiguousarray(local_feats, dtype=np.float32)
    binary_feats = np.ascontiguousarray(binary_feats, dtype=np.float32)
    sparse_idx = np.asarray(sparse_idx)
    W1 = np.ascontiguousarray(W1, dtype=np.float32)
    b1 = np.ascontiguousarray(b1, dtype=np.float32).reshape(1, H)
    W2 = np.ascontiguousarray(W2, dtype=np.float32).reshape(H, 1)
    b2 = np.ascontiguousarray(b2, dtype=np.float32).reshape(1, 1)

    bb = sparse_idx[:, 0].astype(np.int64)
    ii = sparse_idx[:, 1].astype(np.int64)
    jj = sparse_idx[:, 2].astype(np.int64)

    # runtime-sized sparse capacity per (core, batch)
    counts = np.bincount(bb, minlength=B)
    cap_b = max(128, int(-(-counts.max() // 128)) * 128)
    cap = cap_b * BPC

    # IND2: rows 0..99 = [r==j]+[r==i]; row 100 = ones (b1 row)
    cols = np.arange(NN2)
    ind2 = np.zeros((N + 1, NN2), dtype=np.float32)
    np.add.at(ind2, (cols % N, cols), 1.0)
    np.add.at(ind2, (cols // N, cols), 1.0)
    ind2[N, :] = 1.0

    cconst = np.concatenate([b1, W1[H:] * WSCALE], axis=0).astype(cdt)

    in_maps, pos_list = [], []
    for c in range(NCORES):
        sl = slice(c * BPC, c * BPC + BPC)
        binT = binary_feats[sl].transpose(0, 3, 1, 2).reshape(BPC, BIN, NN2)
        rhs112 = np.concatenate(
            [np.broadcast_to(ind2, (BPC, N + 1, NN2)), binT / WSCALE], axis=1)
        if FP8:
            rhs128 = np.concatenate(
                [rhs112, np.zeros((BPC, 16, NN2), np.float32)], axis=1)
            rhs = np.ascontiguousarray(
                rhs128.reshape(BPC, 2, 64, NN2).transpose(0, 2, 1, 3)
            ).astype(f8)
        else:
            rhs = np.ascontiguousarray(rhs112).astype(bf)

        oh = np.zeros((N, cap), dtype=np.float32)
        pos_c = []
        for b in range(BPC):
            gb = c * BPC + b
            pos = np.nonzero(bb == gb)[0]
            assert len(pos) <= cap_b
            colsb = b * cap_b + np.arange(len(pos))
            np.add.at(oh, (ii[pos], colsb), 1.0)
            np.add.at(oh, (jj[pos], colsb), 1.0)
            pos_c.append(pos)
        mc = np.zeros((128, 2103), dtype=np.float32)
        localT = local_feats[sl].transpose(0, 2, 1)          # [BPC, H, N]
        for kt, (k0, kk) in enumerate(((0, 128), (128, 128), (256, 44))):
            mc[0:kk, kt * H:(kt + 1) * H] = W1[k0:k0 + kk, :H]
            for b in range(BPC):
                mc[0:kk, 900 + (b * 3 + kt) * N:900 + (b * 3 + kt + 1) * N] = \
                    localT[b, k0:k0 + kk, :]
            mc[0:kk, 2100 + kt] = W2[k0:k0 + kk, 0]
        for b in range(BPC):
            mc[0:N, 1500 + b * H:1500 + (b + 1) * H] = \
                local_feats[sl][b].reshape(N, H)
        in_maps.append({
            "MC": mc.astype(bf),
            "RHS": rhs,
            "Cconst": cconst,
            "b2": b2,
            "oh": oh.astype(bf),
        })
        pos_list.append(pos_c)
    return in_maps, pos_list, cap_b


def _run(in_maps, cap_b, trace=False):
    from concourse.bass_utils import run_bass_kernel_spmd
    key = (cap_b, FP8)
    if key not in _CACHE:
        _CACHE[key] = _build_nc(cap_b, FP8)
    nc = _CACHE[key]
    _CACHE["last_nc"] = nc
    res = run_bass_kernel_spmd(nc, in_maps, core_ids=list(range(NCORES)),
                               trace=trace)
    return res


def kernel(local_feats, binary_feats, sparse_idx, W1, b1, W2, b2):
    in_maps, pos_list, cap_b = _prep_inputs(
        local_feats, binary_feats, sparse_idx, W1, b1, W2, b2)
    res = _run(in_maps, cap_b)
    E = sparse_idx.shape[0]
    lp_full = np.zeros((E, H), dtype=np.float32)
    gp_full = np.zeros((E, H), dtype=np.float32)
    for c in range(NCORES):
        for b in range(BPC):
            pos = pos_list[c][b]
            r0 = b * cap_b
            lp_full[pos] = res.results[c]["lp"][r0:r0 + len(pos)].astype(
                np.float32)
            gp_full[pos] = res.results[c]["gp"][r0:r0 + len(pos)].astype(
                np.float32)
    return (lp_full, gp_full)
